# revision 4
# baseline (speedup 1.0000x reference)
"""Sparse-attention kernel for 8 trn2 NeuronCores (Bass/Tile).

Math (reference):
    Q = x1 @ Wq.T + bq                       [N1, DIM]
    K = x2 @ Wk.T + bk                       [N2, DIM]
    scores = (Q @ K.T) / sqrt(ITEM)          [N1, N2]
    e = exp(scores) * label_map
    att = e / (sum_j e + 1e-8) * (sum_j label_map / topk + 1e-8)
    out = att @ x2                           [N1, ITEM]

Key transformations used here:
  * Rows of x1/label_map are sharded across 8 cores (512 rows each).
  * scores = Q @ K.T is reassociated as (Q @ Wk) @ x2.T + (Q @ bk) 1^T.
    The (Q @ bk) term is constant per output row i; it scales both the
    numerator e and the denominator sum(e) by exp(c_i), which cancels in
    the normalization (the +1e-8 epsilon makes this inexact only at the
    ~1e-11 relative level since sum(e) is O(1e3)).  So bk drops out and no
    core ever computes the K projection (saving 33.5 GFLOP/core).
  * The 1/sqrt(ITEM) scale and bq bias are folded into the Q epilogue.
  * The per-row normalization a_i is applied to the final out rows, so the
    unnormalized e.T tiles (built via PE transposes) feed the spmm directly.
  * All matmuls run as float32r (fp22-precision reads, 1 cycle/row).
"""

import math

import numpy as np

try:
    import concourse.bass as bass
except ImportError:  # fresh interpreter without the boot path
    import sys

    sys.path.insert(0, "/opt/trn_rl_repo")
    import concourse.bass as bass

import concourse.mybir as mybir
import concourse.tile as tile
from concourse import bacc
from concourse.bass_utils import run_bass_kernel_spmd
from concourse.masks import make_identity

NCORES = 8
F32 = mybir.dt.float32
F32R = mybir.dt.float32r


def _build(S, N2, ITEM, DIMP, denom, topk_f):
    """Build the per-core Bass program.

    S     - x1 rows per core (multiple of 128)
    N2    - x2 rows (multiple of 512)
    ITEM  - feature dim (multiple of 512)
    DIMP  - projection dim padded to a multiple of 128
    denom - sqrt(original ITEM)
    """
    IC = S // 128  # output-row chunks
    JC = N2 // 128  # x2-row chunks (spmm contraction)
    JN = N2 // 512  # 512-wide tiles of the scores free dim
    TC = ITEM // 128  # feature chunks (scores contraction)
    TN = ITEM // 512  # 512-wide tiles of the output free dim
    DC = DIMP // 128  # projection-dim chunks
    Exp = mybir.ActivationFunctionType.Exp
    Mult = mybir.AluOpType.mult
    Add = mybir.AluOpType.add
    X = mybir.AxisListType.X

    nc = bacc.Bacc("TRN2", target_bir_lowering=False, debug=False, num_devices=NCORES)
    x1t = nc.dram_tensor("x1t", [ITEM, S], F32R, kind="ExternalInput")
    wqt = nc.dram_tensor("wqt", [ITEM, DIMP], F32R, kind="ExternalInput")
    wk = nc.dram_tensor("wk", [DIMP, ITEM], F32R, kind="ExternalInput")
    x2t = nc.dram_tensor("x2t", [ITEM, N2], F32R, kind="ExternalInput")
    x2n = nc.dram_tensor("x2n", [N2, ITEM], F32R, kind="ExternalInput")
    lm = nc.dram_tensor("lm", [S, N2], F32, kind="ExternalInput")
    bq2 = nc.dram_tensor("bq2", [128, DC], F32, kind="ExternalInput")
    y = nc.dram_tensor("y", [S, ITEM], F32, kind="ExternalOutput")

    with tile.TileContext(nc) as tc:
        with (
            tc.tile_pool(name="big", bufs=1) as big,
            tc.tile_pool(name="persist", bufs=1) as persist,
            tc.tile_pool(name="stream", bufs=4) as stream,
            tc.tile_pool(name="rhs", bufs=6) as rhspool,
            tc.tile_pool(name="wtile", bufs=6) as wpool,
            tc.tile_pool(name="acc", bufs=4, space="PSUM") as accp,
            tc.tile_pool(name="trp", bufs=4, space="PSUM") as trp,
        ):
            ident = persist.tile([128, 128], F32, tag="ident")
            make_identity(nc, ident[:])
            zbias = persist.tile([128, 1], F32, tag="zbias")
            nc.gpsimd.memset(zbias[:], 0.0)
            bqt = persist.tile([128, DC], F32, tag="bqt")
            nc.sync.dma_start(bqt[:], bq2[:])
            bqs = persist.tile([128, DC], F32, tag="bqs")
            nc.vector.tensor_scalar_mul(bqs[:], bqt[:], 1.0 / denom)

            # phase 1: QT[d, i] = (x1 @ Wq.T + bq) / denom, DIM-major
            x1t_s = big.tile([128, TC, S], F32R, tag="bigA")
            for t in range(TC):
                nc.sync.dma_start(x1t_s[:, t, :], x1t[t * 128 : (t + 1) * 128, :])
            qt_s = persist.tile([128, DC, S], F32R, tag="qt")
            for d in range(DC):
                ps = accp.tile([128, 512], F32, tag="acc")
                for t in range(TC):
                    w = wpool.tile([128, 128], F32R, tag="w")
                    nc.sync.dma_start(
                        w[:], wqt[t * 128 : (t + 1) * 128, d * 128 : (d + 1) * 128]
                    )
                    nc.tensor.matmul(
                        ps[:, :S],
                        w[:],
                        x1t_s[:, t, :],
                        start=(t == 0),
                        stop=(t == TC - 1),
                    )
                nc.vector.tensor_scalar(
                    qt_s[:, d, :], ps[:, :S], 1.0 / denom, bqs[:, d : d + 1],
                    op0=Mult, op1=Add,
                )

            # phase 2: AT[t, i] = sum_d Wk[d, t] * QT[d, i]   (= (Q @ Wk).T)
            at_s = big.tile([128, TC, S], F32R, tag="bigB")
            for t in range(TC):
                ps = accp.tile([128, 512], F32, tag="acc")
                for d in range(DC):
                    w = wpool.tile([128, 128], F32R, tag="w")
                    nc.sync.dma_start(
                        w[:], wk[d * 128 : (d + 1) * 128, t * 128 : (t + 1) * 128]
                    )
                    nc.tensor.matmul(
                        ps[:, :S],
                        w[:],
                        qt_s[:, d, :],
                        start=(d == 0),
                        stop=(d == DC - 1),
                    )
                nc.scalar.copy(at_s[:, t, :], ps[:, :S])

            # phase 3: scores -> exp -> *label -> row-sums -> transpose to eT
            et_s = big.tile([128, JC, IC * 128], F32R, tag="bigA")
            s_parts = persist.tile([128, IC, JN], F32, tag="sparts")
            i_parts = persist.tile([128, IC, JN], F32, tag="iparts")
            for jn in range(JN):
                pss = [
                    accp.tile([128, 512], F32, tag="acc", name=f"ps3_{jn}_{i}")
                    for i in range(IC)
                ]
                for t in range(TC):
                    r = rhspool.tile([128, 512], F32R, tag="rhs")
                    nc.sync.dma_start(
                        r[:], x2t[t * 128 : (t + 1) * 128, jn * 512 : (jn + 1) * 512]
                    )
                    for i in range(IC):
                        nc.tensor.matmul(
                            pss[i][:],
                            at_s[:, t, i * 128 : (i + 1) * 128],
                            r[:],
                            start=(t == 0),
                            stop=(t == TC - 1),
                        )
                for i in range(IC):
                    e = stream.tile([128, 512], F32, tag="e")
                    nc.scalar.activation(e[:], pss[i][:], Exp, bias=zbias[:])
                    l = stream.tile([128, 512], F32, tag="lm")
                    nc.sync.dma_start(
                        l[:], lm[i * 128 : (i + 1) * 128, jn * 512 : (jn + 1) * 512]
                    )
                    nc.vector.reduce_sum(i_parts[:, i, jn : jn + 1], l[:], axis=X)
                    nc.vector.tensor_mul(e[:], e[:], l[:])
                    nc.vector.reduce_sum(s_parts[:, i, jn : jn + 1], e[:], axis=X)
                    for jj in range(4):
                        pt = trp.tile([128, 128], F32, tag="tr")
                        nc.tensor.transpose(
                            pt[:], e[:, jj * 128 : (jj + 1) * 128], ident[:]
                        )
                        nc.scalar.copy(
                            et_s[:, jn * 4 + jj, i * 128 : (i + 1) * 128], pt[:]
                        )

            # a_i = (interactions/topk + 1e-8) / (sum_e + 1e-8)
            s_all = persist.tile([128, IC, 1], F32, tag="sall")
            nc.vector.reduce_sum(s_all[:], s_parts[:], axis=X)
            nc.vector.tensor_scalar_add(s_all[:], s_all[:], 1e-8)
            rec = persist.tile([128, IC, 1], F32, tag="rec")
            nc.vector.reciprocal(rec[:], s_all[:])
            i_all = persist.tile([128, IC, 1], F32, tag="iall")
            nc.vector.reduce_sum(i_all[:], i_parts[:], axis=X)
            nc.vector.tensor_scalar(
                i_all[:], i_all[:], 1.0 / topk_f, 1e-8, op0=Mult, op1=Add
            )
            a_all = persist.tile([128, IC, 1], F32, tag="aall")
            nc.vector.tensor_mul(a_all[:], i_all[:], rec[:])

            # phase 4: out[i, :] = a_i * sum_j eT[j, i] * x2[j, :]
            for n in range(TN):
                ps4 = [
                    accp.tile([128, 512], F32, tag="acc", name=f"ps4_{n}_{i}")
                    for i in range(IC)
                ]
                for j in range(JC):
                    r = rhspool.tile([128, 512], F32R, tag="rhs")
                    nc.sync.dma_start(
                        r[:], x2n[j * 128 : (j + 1) * 128, n * 512 : (n + 1) * 512]
                    )
                    for i in range(IC):
                        nc.tensor.matmul(
                            ps4[i][:],
                            et_s[:, j, i * 128 : (i + 1) * 128],
                            r[:],
                            start=(j == 0),
                            stop=(j == JC - 1),
                        )
                for i in range(IC):
                    o = stream.tile([128, 512], F32, tag="osb")
                    nc.vector.tensor_scalar_mul(o[:], ps4[i][:], a_all[:, i, :])
                    nc.sync.dma_start(
                        y[i * 128 : (i + 1) * 128, n * 512 : (n + 1) * 512], o[:]
                    )

    nc.compile()
    return nc


def _in_maps(x1, x2, label_map, Wq, bq, Wk, DIMP, S):
    ITEM = x1.shape[1]
    DIM = Wq.shape[0]
    DC = DIMP // 128
    wqt = np.zeros((ITEM, DIMP), np.float32)
    wqt[:, :DIM] = Wq.T
    wkp = np.zeros((DIMP, ITEM), np.float32)
    wkp[:DIM] = Wk
    bqp = np.zeros((DIMP,), np.float32)
    bqp[:DIM] = bq
    bq2 = np.ascontiguousarray(bqp.reshape(DC, 128).T)
    x1t = np.ascontiguousarray(x1.T)
    x2t = np.ascontiguousarray(x2.T)
    x2c = np.ascontiguousarray(x2)
    maps = []
    for c in range(NCORES):
        sl = slice(c * S, (c + 1) * S)
        maps.append(
            {
                "x1t": np.ascontiguousarray(x1t[:, sl]),
                "wqt": wqt,
                "wk": wkp,
                "x2t": x2t,
                "x2n": x2c,
                "lm": np.ascontiguousarray(label_map[sl]),
                "bq2": bq2,
            }
        )
    return maps


def _run(x1, x2, label_map, Wq, bq, Wk, bk, topk, trace=False):
    x1 = np.asarray(x1, np.float32)
    x2 = np.asarray(x2, np.float32)
    label_map = np.asarray(label_map, np.float32)
    Wq = np.asarray(Wq, np.float32)
    bq = np.asarray(bq, np.float32)
    Wk = np.asarray(Wk, np.float32)
    N1, ITEM = x1.shape
    N2 = x2.shape[0]
    DIM = Wq.shape[0]
    S = N1 // NCORES
    DIMP = ((DIM + 127) // 128) * 128
    nc = _build(S, N2, ITEM, DIMP, math.sqrt(ITEM), float(topk))
    maps = _in_maps(x1, x2, label_map, Wq, bq, Wk, DIMP, S)
    res = run_bass_kernel_spmd(
        nc, maps, list(range(NCORES)), trace=trace, trace_cores=[0] if trace else None
    )
    out = np.concatenate([res.results[c]["y"] for c in range(NCORES)], axis=0)
    return out.astype(np.float32), res


def kernel(x1, x2, label_map, Wq, bq, Wk, bk, topk):
    out, _ = _run(x1, x2, label_map, Wq, bq, Wk, bk, topk)
    return out


# revision 5
# speedup vs baseline: 1.2072x; 1.2072x over previous
"""Sparse-attention kernel for 8 trn2 NeuronCores (Bass/Tile).

Math (reference):
    Q = x1 @ Wq.T + bq                       [N1, DIM]
    K = x2 @ Wk.T + bk                       [N2, DIM]
    scores = (Q @ K.T) / sqrt(ITEM)          [N1, N2]
    e = exp(scores) * label_map
    att = e / (sum_j e + 1e-8) * (sum_j label_map / topk + 1e-8)
    out = att @ x2                           [N1, ITEM]

Key transformations used here:
  * Rows of x1/label_map are sharded across 8 cores (512 rows each).
  * scores = Q @ K.T is reassociated as (Q @ Wk) @ x2.T + (Q @ bk) 1^T.
    The (Q @ bk) term is constant per output row i; it scales both the
    numerator e and the denominator sum(e) by exp(c_i), which cancels in
    the normalization (the +1e-8 epsilon makes this inexact only at the
    ~1e-11 relative level since sum(e) is O(1e3)).  So bk drops out and no
    core ever computes the K projection (saving 33.5 GFLOP/core).
  * The 1/sqrt(ITEM) scale and bq bias are folded into the Q epilogue.
  * The per-row normalization a_i is applied to the final out rows, so the
    unnormalized e.T tiles (built via PE transposes) feed the spmm directly.
  * Matmul operands are bf16 (fp32 PSUM accumulation).  bf16 weights get
    the fast-weight-load path, which overlaps LDWEIGHTS with the previous
    matmul; 4-byte weights serialize it (~+190 ns per matmul).
  * The big DRAM streams are host-pre-tiled into contiguous [128, x] tile
    blocks so every DMA is one contiguous extent.
"""

import math

import numpy as np

try:
    import concourse.bass as bass
except ImportError:  # fresh interpreter without the boot path
    import sys

    sys.path.insert(0, "/opt/trn_rl_repo")
    import concourse.bass as bass

import ml_dtypes
import concourse.mybir as mybir
import concourse.tile as tile
from concourse import bacc
from concourse.bass_utils import run_bass_kernel_spmd
from concourse.masks import make_identity

NCORES = 8
F32 = mybir.dt.float32
BF16 = mybir.dt.bfloat16
NPBF16 = ml_dtypes.bfloat16


def _build(S, N2, ITEM, DIMP, denom, topk_f):
    """Build the per-core Bass program.

    S     - x1 rows per core (multiple of 128)
    N2    - x2 rows (multiple of 512)
    ITEM  - feature dim (multiple of 512)
    DIMP  - projection dim padded to a multiple of 128
    denom - sqrt(original ITEM)
    """
    IC = S // 128  # output-row chunks
    JC = N2 // 128  # x2-row chunks (spmm contraction)
    JN = N2 // 512  # 512-wide tiles of the scores free dim
    TC = ITEM // 128  # feature chunks (scores contraction)
    TN = ITEM // 512  # 512-wide tiles of the output free dim
    DC = DIMP // 128  # projection-dim chunks
    Exp = mybir.ActivationFunctionType.Exp
    Mult = mybir.AluOpType.mult
    Add = mybir.AluOpType.add
    X = mybir.AxisListType.X

    nc = bacc.Bacc("TRN2", target_bir_lowering=False, debug=False, num_devices=NCORES)
    x1t = nc.dram_tensor("x1t", [TC, 128, S], BF16, kind="ExternalInput")
    wqt = nc.dram_tensor("wqt", [TC, DC, 128, 128], BF16, kind="ExternalInput")
    wk = nc.dram_tensor("wk", [TC, DC, 128, 128], BF16, kind="ExternalInput")
    x2t = nc.dram_tensor("x2t", [TC, JN, 128, 512], BF16, kind="ExternalInput")
    x2n = nc.dram_tensor("x2n", [TN, JC, 128, 512], BF16, kind="ExternalInput")
    lm = nc.dram_tensor("lm", [JN, IC, 128, 512], BF16, kind="ExternalInput")
    bq2 = nc.dram_tensor("bq2", [128, DC], F32, kind="ExternalInput")
    y = nc.dram_tensor("y", [S, ITEM], F32, kind="ExternalOutput")

    with tile.TileContext(nc) as tc:
        with (
            tc.tile_pool(name="big", bufs=1) as big,
            tc.tile_pool(name="persist", bufs=1) as persist,
            tc.tile_pool(name="stream", bufs=4) as stream,
            tc.tile_pool(name="rhs", bufs=6) as rhspool,
            tc.tile_pool(name="wtile", bufs=6) as wpool,
            tc.tile_pool(name="acc", bufs=4, space="PSUM") as accp,
            tc.tile_pool(name="trp", bufs=4, space="PSUM") as trp,
        ):
            ident = persist.tile([128, 128], BF16, tag="ident")
            make_identity(nc, ident[:])
            zbias = persist.tile([128, 1], F32, tag="zbias")
            nc.gpsimd.memset(zbias[:], 0.0)
            bqt = persist.tile([128, DC], F32, tag="bqt")
            nc.sync.dma_start(bqt[:], bq2[:])
            bqs = persist.tile([128, DC], F32, tag="bqs")
            nc.vector.tensor_scalar_mul(bqs[:], bqt[:], 1.0 / denom)

            # phase 1: QT[d, i] = (x1 @ Wq.T + bq) / denom, DIM-major
            x1t_s = big.tile([128, TC, S], BF16, tag="bigA")
            for t in range(TC):
                nc.sync.dma_start(x1t_s[:, t, :], x1t[t])
            qt_s = persist.tile([128, DC, S], BF16, tag="qt")
            for d in range(DC):
                ps = accp.tile([128, 512], F32, tag="acc")
                for t in range(TC):
                    w = wpool.tile([128, 128], BF16, tag="w")
                    nc.sync.dma_start(w[:], wqt[t, d])
                    nc.tensor.matmul(
                        ps[:, :S],
                        w[:],
                        x1t_s[:, t, :],
                        start=(t == 0),
                        stop=(t == TC - 1),
                    )
                nc.vector.tensor_scalar(
                    qt_s[:, d, :], ps[:, :S], 1.0 / denom, bqs[:, d : d + 1],
                    op0=Mult, op1=Add,
                )

            # phase 2: AT[t, i] = sum_d Wk[d, t] * QT[d, i]   (= (Q @ Wk).T)
            at_s = big.tile([128, TC, S], BF16, tag="bigB")
            for t in range(TC):
                ps = accp.tile([128, 512], F32, tag="acc")
                for d in range(DC):
                    w = wpool.tile([128, 128], BF16, tag="w")
                    nc.sync.dma_start(w[:], wk[t, d])
                    nc.tensor.matmul(
                        ps[:, :S],
                        w[:],
                        qt_s[:, d, :],
                        start=(d == 0),
                        stop=(d == DC - 1),
                    )
                nc.scalar.copy(at_s[:, t, :], ps[:, :S])

            # phase 3: scores -> exp -> *label -> row-sums -> transpose to eT
            et_s = big.tile([128, JC, IC * 128], BF16, tag="bigA")
            s_parts = persist.tile([128, IC, JN], F32, tag="sparts")
            i_parts = persist.tile([128, IC, JN], F32, tag="iparts")
            for jn in range(JN):
                pss = [
                    accp.tile([128, 512], F32, tag="acc", name=f"ps3_{jn}_{i}")
                    for i in range(IC)
                ]
                for t in range(TC):
                    r = rhspool.tile([128, 512], BF16, tag="rhs")
                    nc.sync.dma_start(r[:], x2t[t, jn])
                    for i in range(IC):
                        nc.tensor.matmul(
                            pss[i][:],
                            at_s[:, t, i * 128 : (i + 1) * 128],
                            r[:],
                            start=(t == 0),
                            stop=(t == TC - 1),
                        )
                for i in range(IC):
                    e = stream.tile([128, 512], BF16, tag="e")
                    nc.scalar.activation(e[:], pss[i][:], Exp, bias=zbias[:])
                    l = stream.tile([128, 512], BF16, tag="lm")
                    nc.sync.dma_start(l[:], lm[jn, i])
                    nc.vector.reduce_sum(i_parts[:, i, jn : jn + 1], l[:], axis=X)
                    nc.vector.tensor_mul(e[:], e[:], l[:])
                    nc.vector.reduce_sum(s_parts[:, i, jn : jn + 1], e[:], axis=X)
                    for jj in range(4):
                        pt = trp.tile([128, 128], BF16, tag="tr")
                        nc.tensor.transpose(
                            pt[:], e[:, jj * 128 : (jj + 1) * 128], ident[:]
                        )
                        nc.scalar.copy(
                            et_s[:, jn * 4 + jj, i * 128 : (i + 1) * 128], pt[:]
                        )

            # a_i = (interactions/topk + 1e-8) / (sum_e + 1e-8)
            s_all = persist.tile([128, IC, 1], F32, tag="sall")
            nc.vector.reduce_sum(s_all[:], s_parts[:], axis=X)
            nc.vector.tensor_scalar_add(s_all[:], s_all[:], 1e-8)
            rec = persist.tile([128, IC, 1], F32, tag="rec")
            nc.vector.reciprocal(rec[:], s_all[:])
            i_all = persist.tile([128, IC, 1], F32, tag="iall")
            nc.vector.reduce_sum(i_all[:], i_parts[:], axis=X)
            nc.vector.tensor_scalar(
                i_all[:], i_all[:], 1.0 / topk_f, 1e-8, op0=Mult, op1=Add
            )
            a_all = persist.tile([128, IC, 1], F32, tag="aall")
            nc.vector.tensor_mul(a_all[:], i_all[:], rec[:])

            # phase 4: out[i, :] = a_i * sum_j eT[j, i] * x2[j, :]
            for n in range(TN):
                ps4 = [
                    accp.tile([128, 512], F32, tag="acc", name=f"ps4_{n}_{i}")
                    for i in range(IC)
                ]
                for j in range(JC):
                    r = rhspool.tile([128, 512], BF16, tag="rhs")
                    nc.sync.dma_start(r[:], x2n[n, j])
                    for i in range(IC):
                        nc.tensor.matmul(
                            ps4[i][:],
                            et_s[:, j, i * 128 : (i + 1) * 128],
                            r[:],
                            start=(j == 0),
                            stop=(j == JC - 1),
                        )
                for i in range(IC):
                    o = stream.tile([128, 512], F32, tag="osb")
                    nc.vector.tensor_scalar_mul(o[:], ps4[i][:], a_all[:, i, :])
                    nc.sync.dma_start(
                        y[i * 128 : (i + 1) * 128, n * 512 : (n + 1) * 512], o[:]
                    )

    nc.compile()
    return nc


def _tile4(a, p, f):
    """[R, C] -> [R//p, C//f, p, f] contiguous tile blocks."""
    R, C = a.shape
    return np.ascontiguousarray(
        a.reshape(R // p, p, C // f, f).transpose(0, 2, 1, 3)
    )


def _in_maps(x1, x2, label_map, Wq, bq, Wk, DIMP, S):
    ITEM = x1.shape[1]
    N2 = x2.shape[0]
    DIM = Wq.shape[0]
    DC = DIMP // 128
    wqp = np.zeros((DIMP, ITEM), NPBF16)
    wqp[:DIM] = Wq.astype(NPBF16)
    wkp = np.zeros((DIMP, ITEM), NPBF16)
    wkp[:DIM] = Wk.astype(NPBF16)
    bqp = np.zeros((DIMP,), np.float32)
    bqp[:DIM] = bq
    bq2 = np.ascontiguousarray(bqp.reshape(DC, 128).T)

    x1b = x1.astype(NPBF16)
    x2b = x2.astype(NPBF16)
    # wqt[t, d] = Wq.T[t-chunk, d-chunk];  wk[t, d] = Wk[d-chunk, t-chunk]
    wqt = _tile4(np.ascontiguousarray(wqp.T), 128, 128)
    wkt = np.ascontiguousarray(_tile4(wkp, 128, 128).transpose(1, 0, 2, 3))
    x2tb = _tile4(np.ascontiguousarray(x2b.T), 128, 512)
    x2nb = np.ascontiguousarray(_tile4(x2b, 128, 512).transpose(1, 0, 2, 3))
    maps = []
    for c in range(NCORES):
        sl = slice(c * S, (c + 1) * S)
        lmb = _tile4(label_map[sl].astype(NPBF16), 128, 512)
        maps.append(
            {
                "x1t": np.ascontiguousarray(x1b[sl].T).reshape(-1, 128, S),
                "wqt": wqt,
                "wk": wkt,
                "x2t": x2tb,
                "x2n": x2nb,
                "lm": np.ascontiguousarray(lmb.transpose(1, 0, 2, 3)),
                "bq2": bq2,
            }
        )
    return maps


def _run(x1, x2, label_map, Wq, bq, Wk, bk, topk, trace=False):
    x1 = np.asarray(x1, np.float32)
    x2 = np.asarray(x2, np.float32)
    label_map = np.asarray(label_map, np.float32)
    Wq = np.asarray(Wq, np.float32)
    bq = np.asarray(bq, np.float32)
    Wk = np.asarray(Wk, np.float32)
    N1, ITEM = x1.shape
    N2 = x2.shape[0]
    DIM = Wq.shape[0]
    S = N1 // NCORES
    DIMP = ((DIM + 127) // 128) * 128
    nc = _build(S, N2, ITEM, DIMP, math.sqrt(ITEM), float(topk))
    maps = _in_maps(x1, x2, label_map, Wq, bq, Wk, DIMP, S)
    res = run_bass_kernel_spmd(
        nc, maps, list(range(NCORES)), trace=trace, trace_cores=[0] if trace else None
    )
    out = np.concatenate([res.results[c]["y"] for c in range(NCORES)], axis=0)
    return out.astype(np.float32), res


def kernel(x1, x2, label_map, Wq, bq, Wk, bk, topk):
    out, _ = _run(x1, x2, label_map, Wq, bq, Wk, bk, topk)
    return out
